# revision 10
# baseline (speedup 1.0000x reference)
"""Trainium2 Bass kernel for a sigmoid-scored attention decode step with KV cache.

Reference computation (all fp32):
    q = W_query @ x.T ; k = W_key @ x.T ; v = W_value @ x.T          # [4096, 1]
    K = [K_cache | k] ; V = [V_cache | v]                            # [4096, 8193]
    a = sigmoid((q.T @ K) / 64)                                      # [1, 8193]
    z = V @ a.T                                                      # [4096, 1]

Sharding: rows of W_q/W_k/K_cache/V_cache are split across 8 NeuronCores (512
rows each). Each core computes its q/k shard and partial scores over its 512
rows of K; the partial score vectors are summed across cores; sigmoid + the
V-weighted sum are then local per shard. Host slices inputs, casts to bf16
(gate is 2e-2; the problem is HBM-bound so halving traffic dominates), and
pre-transposes the V and W_value shards so the z-phase and the current-step v
run on the tensor engine.

v5 (vs v4 178us / v3 200us / v2 210us / v1 233us):
  - the cross-core score reduction is a hand-rolled all-to-all over
    remote_dma_broadcast (one single-dest send per peer, XOR-relative
    addressing) + 7 local DVE adds, replacing the ncfw AllReduce whose
    fixed cost (trigger->output ~50-65us, after a ~45-65us init) dominated
    v3/v4. Partial scores are exchanged in an interleaved [128, 65] layout
    (one xbar transpose-DMA via a DRAM bounce) so the reduce, sigmoid, and
    the PE z-phase lhsT all use the same tile.
  - v = W_v @ x moved to PE: host supplies W_v^T; x is transpose-loaded as
    [128, 32] chunk-columns; 32 accumulating matmuls give v as a [1, 512]
    row directly (drops 18us of DVE TTRs + the DRAM bounce of v).
"""

import sys

for _p in ("/opt/trn_rl_repo", "/root/.axon_site/_ro/trn_rl_repo"):
    if _p not in sys.path:
        sys.path.append(_p)

import ml_dtypes
import numpy as np

import concourse.bacc as bacc
import concourse.tile as tile
from concourse import mybir
from concourse.bass_utils import run_bass_kernel_spmd
from concourse.dve_ops import TENSOR_TENSOR_REDUCE

N_CORES = 8
E = 4096          # embedding dim (contraction for q/k/v)
D = 4096          # output dim
T = 8192          # cached timesteps
F32 = mybir.dt.float32
BF16 = mybir.dt.bfloat16
BF16_NP = ml_dtypes.bfloat16


def build(n_cores=N_CORES, e=E, d_sh=D // N_CORES, t=T):
    nd = d_sh // 128             # partition-chunks per core (4)
    half = t // 2                # score columns per K column-group (4096)
    nr = t // 128                # t-chunks for the PE z phase (64)
    wex = 80                     # exchange tile cols (>= 65, multiple of 16)
    payw = wex * 128             # staging row width (10240)
    ne = e // 128                # e-chunks for the PE v row (32)

    nc = bacc.Bacc("TRN2", target_bir_lowering=False, debug=False,
                   num_devices=n_cores)
    x_d = nc.dram_tensor("x", [1, e], BF16, kind="ExternalInput").ap()
    wq_d = nc.dram_tensor("wq", [d_sh, e], BF16, kind="ExternalInput").ap()
    wk_d = nc.dram_tensor("wk", [d_sh, e], BF16, kind="ExternalInput").ap()
    wvt_d = nc.dram_tensor("wvt", [e, d_sh], BF16, kind="ExternalInput").ap()
    kc_d = nc.dram_tensor("kc", [d_sh, t], BF16, kind="ExternalInput").ap()
    vct_d = nc.dram_tensor("vct", [t, d_sh], BF16, kind="ExternalInput").ap()
    z_d = nc.dram_tensor("z", [1, d_sh], F32, kind="ExternalOutput").ap()

    def chunked(src):
        # [n*128, w] DRAM region -> [128, n, w]: block c holds rows
        # 128c..128c+127. Paired with a [p (c t) -> p c t] view of the tile.
        return src.rearrange("(c p) t -> p c t", p=128)

    def as3d(tile_ap, w):
        return tile_ap.rearrange("p (c t) -> p c t", t=w)

    x_recv = nc.alloc_semaphore("x_recv")
    x_sent = nc.alloc_semaphore("x_sent")

    with tile.TileContext(nc) as tc:
        with (
            tc.tile_pool(name="stream", bufs=6) as sp,       # streamed 2MB tiles
            tc.tile_pool(name="scratch", bufs=1) as scp,     # ttr elementwise outs
            tc.tile_pool(name="keep", bufs=1) as kp,         # persistent tiles
            tc.tile_pool(name="acc", bufs=8) as accp,        # [128,1] accumulators
            tc.tile_pool(name="dram", bufs=1, space="DRAM") as dramp,
        ):
            # --- x: broadcast across partitions + interleaved chunk-columns ---
            bx = kp.tile([128, e], BF16, tag="bx", name="bx")
            nc.sync.dma_start(bx[:], x_d[0:1, :].partition_broadcast(128))
            x_c = kp.tile([128, ne], BF16, tag="xc", name="x_c")
            nc.scalar.dma_start_transpose(
                x_c[:], x_d[0:1, :].rearrange("1 (r p) -> r p", p=128))

            ones_col = kp.tile([128, 1], BF16, tag="onesc", name="ones_col")
            nc.vector.memset(ones_col[:], 1.0)
            # pre-warm the sigmoid ACT table so the load is off the critical path
            warm = kp.tile([1, 1], BF16, tag="warm", name="warm")
            nc.vector.memset(warm[:], 0.0)
            nc.scalar.activation(warm[:], warm[:],
                                 mybir.ActivationFunctionType.Sigmoid,
                                 scale=1.0 / 64.0)

            # partial scores staging row (bf16): [0,t) cache scores, t = qk,
            # rest zero
            s_sb = kp.tile([1, payw], BF16, tag="s", name="s_sb")
            nc.vector.memset(s_sb[0:1, t + 1:payw], 0.0)

            qkv_all = kp.tile([128, 2 * nd], BF16, tag="qkv", name="qkv_all")

            def w_matvec(w_dram, col0):
                # two [128, 2*e] transfers; TTR against bx per col block
                for k in range(2):
                    wt = sp.tile([128, 2 * e], BF16, tag="big",
                                 name=f"wt{col0}_{k}")
                    nc.sync.dma_start(as3d(wt[:], e),
                                      chunked(w_dram[256 * k:256 * (k + 1), :]))
                    for c in range(2):
                        sc = scp.tile([128, e], BF16, tag="sc",
                                      name=f"wsc{col0}_{k}_{c}")
                        nc.vector._custom_dve(
                            TENSOR_TENSOR_REDUCE, out=sc[:],
                            in0=wt[:, e * c:e * (c + 1)], in1=bx[:],
                            s0=0.0, s1=1.0,
                            accum_out=qkv_all[:, col0 + 2 * k + c:col0 + 2 * k + c + 1],
                        )

            w_matvec(wq_d, 0)        # q in cols 0..nd-1
            w_matvec(wk_d, nd)       # k in cols nd..2nd-1

            psq_ctx = tc.tile_pool(name="psq", bufs=1, space="PSUM")
            psq = psq_ctx.__enter__()
            psp_ctx = tc.tile_pool(name="ps", bufs=3, space="PSUM")
            psp = psp_ctx.__enter__()
            psv_ctx = tc.tile_pool(name="psv", bufs=1, space="PSUM")
            psv = psv_ctx.__enter__()

            # --- appended-column partial score q.k (early: it gates the
            # --- exchange together with the score casts) ---
            qk_el = scp.tile([128, nd], BF16, tag="sc", name="qk_el")
            qk_part = accp.tile([128, 1], BF16, tag="acc", name="qk_part")
            nc.vector._custom_dve(
                TENSOR_TENSOR_REDUCE, out=qk_el[:], in0=qkv_all[:, 0:nd],
                in1=qkv_all[:, nd:2 * nd], s0=0.0, s1=1.0,
                accum_out=qk_part[:],
            )
            qk_ps = psq.tile([1, 512], F32, tag="psq", name="qk_ps")
            nc.tensor.matmul(qk_ps[0:1, 0:1], lhsT=ones_col[:],
                             rhs=qk_part[:], start=True, stop=True)
            nc.vector.tensor_copy(s_sb[0:1, t:t + 1], qk_ps[0:1, 0:1])

            # --- partial scores: [1,1024] PSUM tiles, 4 per column group ---
            score_tiles = {}
            for g in range(2):
                for k in range(2):
                    kt = sp.tile([128, 2 * half], BF16, tag="big",
                                 name=f"kt{g}_{k}")
                    nc.sync.dma_start(
                        as3d(kt[:], half),
                        chunked(kc_d[256 * k:256 * (k + 1),
                                     half * g:half * (g + 1)]))
                    for ci in range(2):
                        c = 2 * k + ci
                        for i in range(4):
                            if k == 0 and ci == 0:
                                score_tiles[(g, i)] = psp.tile(
                                    [1, 1024], F32, tag="ps", name=f"ps{g}_{i}")
                            ps = score_tiles[(g, i)]
                            for j in range(2):
                                lo = half * ci + 1024 * i + 512 * j
                                nc.tensor.matmul(
                                    ps[0:1, 512 * j:512 * (j + 1)],
                                    lhsT=qkv_all[:, c:c + 1],
                                    rhs=kt[:, lo:lo + 512],
                                    start=(c == 0), stop=(c == nd - 1),
                                )
                # copy-cast f32 PSUM -> bf16 staging
                for i in range(4):
                    nc.vector.tensor_copy(
                        s_sb[0:1, half * g + 1024 * i:half * g + 1024 * (i + 1)],
                        score_tiles[(g, i)][:])

            # --- exchange: bounce to DRAM, transpose-load interleaved, send to
            # --- all 7 peers, reduce locally ---
            sc_d = dramp.tile([1, payw], BF16, tag="sc_d", name="sc_d")
            nc.gpsimd.dma_start(sc_d[:], s_sb[:])
            a_x = kp.tile([128, wex], BF16, tag="ax", name="a_x")
            nc.scalar.dma_start_transpose(
                a_x[:], sc_d[0:1, :].rearrange("1 (r p) -> r p", p=128))

            recvs = []
            for j in range(1, n_cores):
                rv = kp.tile([128, wex], BF16, tag=f"rv{j}", name=f"rv{j}")
                recvs.append(rv)
                rdests = [None] * 8
                rdests[j] = (0, j)
                nc.gpsimd.remote_dma_broadcast(
                    rv[:], a_x[:], remote_sem=x_recv, local_sem=x_sent,
                    rdests=rdests,
                )
            nc.gpsimd.trigger_dma(count=None)

            # accumulate the 7 peer contributions (ping-pong). The wait for
            # x_recv >= 14 is attached to the FIRST add post-scheduling (the
            # Tile scheduling simulator cannot model externally-satisfied
            # semaphores and would report a deadlock).
            asums = [kp.tile([128, wex], BF16, tag=f"asum{i}", name=f"asum{i}")
                     for i in range(2)]
            cur = a_x
            first_add = None
            for idx, rv in enumerate(recvs):
                nxt = asums[idx % 2]
                bi = nc.vector.scalar_tensor_tensor(
                    out=nxt[:], in0=cur[:], scalar=1.0, in1=rv[:],
                    op0=mybir.AluOpType.mult, op1=mybir.AluOpType.add,
                )
                if first_add is None:
                    first_add = bi
                cur = nxt
            a_fin = cur
            nc.scalar.activation(a_fin[:, 0:65], a_fin[:, 0:65],
                                 mybir.ActivationFunctionType.Sigmoid,
                                 scale=1.0 / 64.0)

            # --- v = W_v @ x as a [1, 512] row on PE ---
            v_ps = psv.tile([1, d_sh], F32, tag="psv", name="v_ps")
            for k in range(2):
                wvt = sp.tile([128, 16 * d_sh], BF16, tag="big",
                              name=f"wvt{k}")
                nc.sync.dma_start(
                    as3d(wvt[:], d_sh),
                    chunked(wvt_d[2048 * k:2048 * (k + 1), :]))
                for m in range(16):
                    nc.tensor.matmul(
                        v_ps[:], lhsT=x_c[:, 16 * k + m:16 * k + m + 1],
                        rhs=wvt[:, 512 * m:512 * (m + 1)],
                        start=(k == 0 and m == 0), stop=(k == 1 and m == 15),
                    )
            v_row = kp.tile([1, d_sh], BF16, tag="vrow", name="v_row")
            nc.vector.tensor_copy(v_row[:], v_ps[:])

            psv_ctx.__exit__(None, None, None)
            psp_ctx.__exit__(None, None, None)
            psq_ctx.__exit__(None, None, None)

            # --- z = V @ a on PE: 64 accumulating matmuls + appended column ---
            with tc.tile_pool(name="ps2", bufs=1, space="PSUM") as psp2:
                z_ps = psp2.tile([1, d_sh], F32, tag="zps", name="z_ps")
                for k in range(4):
                    vt = sp.tile([128, 16 * d_sh], BF16, tag="big",
                                 name=f"vt{k}")
                    nc.sync.dma_start(
                        as3d(vt[:], d_sh),
                        chunked(vct_d[2048 * k:2048 * (k + 1), :]))
                    for r in range(16):
                        nc.tensor.matmul(
                            z_ps[:], lhsT=a_fin[:, 16 * k + r:16 * k + r + 1],
                            rhs=vt[:, 512 * r:512 * (r + 1)],
                            start=(k == 0 and r == 0), stop=False,
                        )
                nc.tensor.matmul(z_ps[:], lhsT=a_fin[0:1, 64:65], rhs=v_row[:],
                                 start=False, stop=True)

                z_sb = kp.tile([1, d_sh], F32, tag="zsb", name="z_sb")
                nc.vector.tensor_copy(z_sb[:], z_ps[:])
                nc.gpsimd.dma_start(z_d[:], z_sb[:])

    # Attach the external-semaphore wait to the first reduce add, now that
    # Tile scheduling is done: HW blocks the DVE queue here until all 7
    # remote writes have landed (each bumps x_recv by 2).
    ins = first_add.ins
    si = ins.sync_info
    ow = list(si.on_wait) if si is not None else []
    ou = list(si.on_update) if si is not None else []
    ow.append(mybir.SyncWait(sync_type="semaphore", id=x_recv.num,
                             wait_mode="sem-ge-imm",
                             wait_value=2 * (n_cores - 1)))
    ins.sync_info = mybir.SyncInfo(on_wait=ow, on_update=ou)

    nc.compile()
    return nc


def make_in_maps(inputs, n_cores=N_CORES, d_sh=D // N_CORES):
    def bf(a):
        return np.ascontiguousarray(np.asarray(a, np.float32).astype(BF16_NP))

    x = bf(inputs["x"])
    wq = bf(inputs["W_query"])
    wk = bf(inputs["W_key"])
    wv = bf(inputs["W_value"])
    kc = bf(inputs["K_cache"])
    vc = bf(inputs["V_cache"])
    in_maps = []
    for i in range(n_cores):
        r0, r1 = d_sh * i, d_sh * (i + 1)
        in_maps.append({
            "x": x,
            "wq": np.ascontiguousarray(wq[r0:r1]),
            "wk": np.ascontiguousarray(wk[r0:r1]),
            "wvt": np.ascontiguousarray(wv[r0:r1].T),
            "kc": np.ascontiguousarray(kc[r0:r1]),
            "vct": np.ascontiguousarray(vc[r0:r1].T),
        })
    return in_maps


def unshard(per_core_z, d_sh=D // N_CORES):
    shards = [np.asarray(zi).reshape(d_sh, 1) for zi in per_core_z]
    return np.concatenate(shards, axis=0).astype(np.float32)


_NC_CACHE = None


def kernel(x, W_query, W_key, W_value, K_cache, V_cache):
    global _NC_CACHE
    if _NC_CACHE is None:
        _NC_CACHE = build()
    nc = _NC_CACHE
    in_maps = make_in_maps(dict(x=x, W_query=W_query, W_key=W_key,
                                W_value=W_value, K_cache=K_cache,
                                V_cache=V_cache))
    res = run_bass_kernel_spmd(nc, in_maps, core_ids=list(range(N_CORES)))
    return unshard([res.results[i]["z"] for i in range(N_CORES)])


# revision 11
# speedup vs baseline: 47.6930x; 47.6930x over previous
"""Trainium2 Bass kernel for a sigmoid-scored attention decode step with KV cache.

Reference computation (all fp32):
    q = W_query @ x.T ; k = W_key @ x.T ; v = W_value @ x.T          # [4096, 1]
    K = [K_cache | k] ; V = [V_cache | v]                            # [4096, 8193]
    a = sigmoid((q.T @ K) / 64)                                      # [1, 8193]
    z = V @ a.T                                                      # [4096, 1]

Sharding: rows of W_q/W_k/K_cache/V_cache are split across 8 NeuronCores (512
rows each). Each core computes its q/k shard and partial scores over its 512
rows of K; the partial score vectors are summed across cores; sigmoid + the
V-weighted sum are then local per shard. Host slices inputs, casts to bf16
(gate is 2e-2; the problem is HBM-bound so halving traffic dominates), and
pre-transposes the V and W_value shards so the z-phase and the current-step v
run on the tensor engine.

v5 (vs v4 178us / v3 200us / v2 210us / v1 233us):
  - the cross-core score reduction is a hand-rolled all-to-all over
    remote_dma_broadcast (one single-dest send per peer, XOR-relative
    addressing) + 7 local DVE adds, replacing the ncfw AllReduce whose
    fixed cost (trigger->output ~50-65us, after a ~45-65us init) dominated
    v3/v4. Partial scores are exchanged in an interleaved [128, 65] layout
    (one xbar transpose-DMA via a DRAM bounce) so the reduce, sigmoid, and
    the PE z-phase lhsT all use the same tile.
  - v = W_v @ x moved to PE: host supplies W_v^T; x is transpose-loaded as
    [128, 32] chunk-columns; 32 accumulating matmuls give v as a [1, 512]
    row directly (drops 18us of DVE TTRs + the DRAM bounce of v).
"""

import sys

for _p in ("/opt/trn_rl_repo", "/root/.axon_site/_ro/trn_rl_repo"):
    if _p not in sys.path:
        sys.path.append(_p)

import ml_dtypes
import numpy as np

import concourse.bacc as bacc
import concourse.tile as tile
from concourse import mybir
from concourse.bass_utils import run_bass_kernel_spmd
from concourse.dve_ops import TENSOR_TENSOR_REDUCE

N_CORES = 8
E = 4096          # embedding dim (contraction for q/k/v)
D = 4096          # output dim
T = 8192          # cached timesteps
F32 = mybir.dt.float32
BF16 = mybir.dt.bfloat16
BF16_NP = ml_dtypes.bfloat16


def build(n_cores=N_CORES, e=E, d_sh=D // N_CORES, t=T):
    nd = d_sh // 128             # partition-chunks per core (4)
    half = t // 2                # score columns per K column-group (4096)
    nr = t // 128                # t-chunks for the PE z phase (64)
    wex = 80                     # exchange tile cols (>= 65, multiple of 16)
    payw = wex * 128             # staging row width (10240)
    ne = e // 128                # e-chunks for the PE v row (32)

    nc = bacc.Bacc("TRN2", target_bir_lowering=False, debug=False,
                   num_devices=n_cores)
    x_d = nc.dram_tensor("x", [1, e], BF16, kind="ExternalInput").ap()
    wq_d = nc.dram_tensor("wq", [d_sh, e], BF16, kind="ExternalInput").ap()
    wk_d = nc.dram_tensor("wk", [d_sh, e], BF16, kind="ExternalInput").ap()
    wvt_d = nc.dram_tensor("wvt", [e, d_sh], BF16, kind="ExternalInput").ap()
    kc_d = nc.dram_tensor("kc", [d_sh, t], BF16, kind="ExternalInput").ap()
    vct_d = nc.dram_tensor("vct", [t, d_sh], BF16, kind="ExternalInput").ap()
    z_d = nc.dram_tensor("z", [1, d_sh + 8], F32, kind="ExternalOutput").ap()

    def chunked(src):
        # [n*128, w] DRAM region -> [128, n, w]: block c holds rows
        # 128c..128c+127. Paired with a [p (c t) -> p c t] view of the tile.
        return src.rearrange("(c p) t -> p c t", p=128)

    def as3d(tile_ap, w):
        return tile_ap.rearrange("p (c t) -> p c t", t=w)

    x_recv = nc.alloc_semaphore("x_recv")
    x_sent = nc.alloc_semaphore("x_sent")

    with tile.TileContext(nc) as tc:
        with (
            tc.tile_pool(name="stream", bufs=6) as sp,       # streamed 2MB tiles
            tc.tile_pool(name="scratch", bufs=1) as scp,     # ttr elementwise outs
            tc.tile_pool(name="keep", bufs=1) as kp,         # persistent tiles
            tc.tile_pool(name="acc", bufs=8) as accp,        # [128,1] accumulators
            tc.tile_pool(name="dram", bufs=1, space="DRAM") as dramp,
        ):
            # --- warm-up collective: presence of a collective switches NRT
            # --- into synchronized multi-core launch (without it the 8 cores
            # --- start milliseconds apart and the score exchange eats the
            # --- skew); its zero output is appended to the z output so it
            # --- cannot be dead-code-eliminated and gates nothing.
            w_sb = kp.tile([1, 8], F32, tag="warmsb", name="w_sb")
            nc.vector.memset(w_sb[:], 0.0)
            cc_w_in = dramp.tile([1, 8], F32, tag="cc_w_in", name="cc_w_in")
            cc_w_out = dramp.tile([1, 8], F32, tag="cc_w_out", name="cc_w_out")
            nc.gpsimd.dma_start(cc_w_in[:], w_sb[:])
            nc.gpsimd.collective_compute(
                "AllReduce", mybir.AluOpType.add,
                replica_groups=[list(range(n_cores))],
                ins=[cc_w_in.opt()], outs=[cc_w_out.opt()],
            )

            # --- x: broadcast across partitions + interleaved chunk-columns ---
            bx = kp.tile([128, e], BF16, tag="bx", name="bx")
            nc.sync.dma_start(bx[:], x_d[0:1, :].partition_broadcast(128))
            x_c = kp.tile([128, ne], BF16, tag="xc", name="x_c")
            nc.scalar.dma_start_transpose(
                x_c[:], x_d[0:1, :].rearrange("1 (r p) -> r p", p=128))

            ones_col = kp.tile([128, 1], BF16, tag="onesc", name="ones_col")
            nc.vector.memset(ones_col[:], 1.0)
            # pre-warm the sigmoid ACT table so the load is off the critical path
            warm = kp.tile([1, 1], BF16, tag="warm", name="warm")
            nc.vector.memset(warm[:], 0.0)
            nc.scalar.activation(warm[:], warm[:],
                                 mybir.ActivationFunctionType.Sigmoid,
                                 scale=1.0 / 64.0)

            # partial scores staging row (bf16): [0,t) cache scores, t = qk,
            # rest zero
            s_sb = kp.tile([1, payw], BF16, tag="s", name="s_sb")
            nc.vector.memset(s_sb[0:1, t + 1:payw], 0.0)

            qkv_all = kp.tile([128, 2 * nd], BF16, tag="qkv", name="qkv_all")

            def w_matvec(w_dram, col0):
                # two [128, 2*e] transfers; TTR against bx per col block
                for k in range(2):
                    wt = sp.tile([128, 2 * e], BF16, tag="big",
                                 name=f"wt{col0}_{k}")
                    nc.sync.dma_start(as3d(wt[:], e),
                                      chunked(w_dram[256 * k:256 * (k + 1), :]))
                    for c in range(2):
                        sc = scp.tile([128, e], BF16, tag="sc",
                                      name=f"wsc{col0}_{k}_{c}")
                        nc.vector._custom_dve(
                            TENSOR_TENSOR_REDUCE, out=sc[:],
                            in0=wt[:, e * c:e * (c + 1)], in1=bx[:],
                            s0=0.0, s1=1.0,
                            accum_out=qkv_all[:, col0 + 2 * k + c:col0 + 2 * k + c + 1],
                        )

            w_matvec(wq_d, 0)        # q in cols 0..nd-1
            w_matvec(wk_d, nd)       # k in cols nd..2nd-1

            psq_ctx = tc.tile_pool(name="psq", bufs=1, space="PSUM")
            psq = psq_ctx.__enter__()
            psp_ctx = tc.tile_pool(name="ps", bufs=3, space="PSUM")
            psp = psp_ctx.__enter__()
            psv_ctx = tc.tile_pool(name="psv", bufs=1, space="PSUM")
            psv = psv_ctx.__enter__()

            # --- appended-column partial score q.k (early: it gates the
            # --- exchange together with the score casts) ---
            qk_el = scp.tile([128, nd], BF16, tag="sc", name="qk_el")
            qk_part = accp.tile([128, 1], BF16, tag="acc", name="qk_part")
            nc.vector._custom_dve(
                TENSOR_TENSOR_REDUCE, out=qk_el[:], in0=qkv_all[:, 0:nd],
                in1=qkv_all[:, nd:2 * nd], s0=0.0, s1=1.0,
                accum_out=qk_part[:],
            )
            qk_ps = psq.tile([1, 512], F32, tag="psq", name="qk_ps")
            nc.tensor.matmul(qk_ps[0:1, 0:1], lhsT=ones_col[:],
                             rhs=qk_part[:], start=True, stop=True)
            nc.vector.tensor_copy(s_sb[0:1, t:t + 1], qk_ps[0:1, 0:1])

            # --- partial scores: [1,1024] PSUM tiles, 4 per column group ---
            score_tiles = {}
            for g in range(2):
                for k in range(2):
                    kt = sp.tile([128, 2 * half], BF16, tag="big",
                                 name=f"kt{g}_{k}")
                    nc.sync.dma_start(
                        as3d(kt[:], half),
                        chunked(kc_d[256 * k:256 * (k + 1),
                                     half * g:half * (g + 1)]))
                    for ci in range(2):
                        c = 2 * k + ci
                        for i in range(4):
                            if k == 0 and ci == 0:
                                score_tiles[(g, i)] = psp.tile(
                                    [1, 1024], F32, tag="ps", name=f"ps{g}_{i}")
                            ps = score_tiles[(g, i)]
                            for j in range(2):
                                lo = half * ci + 1024 * i + 512 * j
                                nc.tensor.matmul(
                                    ps[0:1, 512 * j:512 * (j + 1)],
                                    lhsT=qkv_all[:, c:c + 1],
                                    rhs=kt[:, lo:lo + 512],
                                    start=(c == 0), stop=(c == nd - 1),
                                )
                # copy-cast f32 PSUM -> bf16 staging
                for i in range(4):
                    nc.vector.tensor_copy(
                        s_sb[0:1, half * g + 1024 * i:half * g + 1024 * (i + 1)],
                        score_tiles[(g, i)][:])

            # --- exchange: bounce to DRAM, transpose-load interleaved, send to
            # --- all 7 peers, reduce locally ---
            sc_d = dramp.tile([1, payw], BF16, tag="sc_d", name="sc_d")
            nc.gpsimd.dma_start(sc_d[:], s_sb[:])
            a_x = kp.tile([128, wex], BF16, tag="ax", name="a_x")
            nc.scalar.dma_start_transpose(
                a_x[:], sc_d[0:1, :].rearrange("1 (r p) -> r p", p=128))

            recvs = []
            for j in range(1, n_cores):
                rv = kp.tile([128, wex], BF16, tag=f"rv{j}", name=f"rv{j}")
                recvs.append(rv)
                rdests = [None] * 8
                rdests[j] = (0, j)
                nc.gpsimd.remote_dma_broadcast(
                    rv[:], a_x[:], remote_sem=x_recv, local_sem=x_sent,
                    rdests=rdests,
                )
            nc.gpsimd.trigger_dma(count=None)

            # accumulate the 7 peer contributions (ping-pong). The wait for
            # x_recv >= 14 is attached to the FIRST add post-scheduling (the
            # Tile scheduling simulator cannot model externally-satisfied
            # semaphores and would report a deadlock).
            asums = [kp.tile([128, wex], BF16, tag=f"asum{i}", name=f"asum{i}")
                     for i in range(2)]
            cur = a_x
            first_add = None
            for idx, rv in enumerate(recvs):
                nxt = asums[idx % 2]
                bi = nc.vector.scalar_tensor_tensor(
                    out=nxt[:], in0=cur[:], scalar=1.0, in1=rv[:],
                    op0=mybir.AluOpType.mult, op1=mybir.AluOpType.add,
                )
                if first_add is None:
                    first_add = bi
                cur = nxt
            a_fin = cur
            nc.scalar.activation(a_fin[:, 0:65], a_fin[:, 0:65],
                                 mybir.ActivationFunctionType.Sigmoid,
                                 scale=1.0 / 64.0)

            # --- v = W_v @ x as a [1, 512] row on PE ---
            v_ps = psv.tile([1, d_sh], F32, tag="psv", name="v_ps")
            for k in range(2):
                wvt = sp.tile([128, 16 * d_sh], BF16, tag="big",
                              name=f"wvt{k}")
                nc.sync.dma_start(
                    as3d(wvt[:], d_sh),
                    chunked(wvt_d[2048 * k:2048 * (k + 1), :]))
                for m in range(16):
                    nc.tensor.matmul(
                        v_ps[:], lhsT=x_c[:, 16 * k + m:16 * k + m + 1],
                        rhs=wvt[:, 512 * m:512 * (m + 1)],
                        start=(k == 0 and m == 0), stop=(k == 1 and m == 15),
                    )
            v_row = kp.tile([1, d_sh], BF16, tag="vrow", name="v_row")
            nc.vector.tensor_copy(v_row[:], v_ps[:])

            psv_ctx.__exit__(None, None, None)
            psp_ctx.__exit__(None, None, None)
            psq_ctx.__exit__(None, None, None)

            # --- z = V @ a on PE: 64 accumulating matmuls + appended column ---
            with tc.tile_pool(name="ps2", bufs=1, space="PSUM") as psp2:
                z_ps = psp2.tile([1, d_sh], F32, tag="zps", name="z_ps")
                for k in range(4):
                    vt = sp.tile([128, 16 * d_sh], BF16, tag="big",
                                 name=f"vt{k}")
                    nc.sync.dma_start(
                        as3d(vt[:], d_sh),
                        chunked(vct_d[2048 * k:2048 * (k + 1), :]))
                    for r in range(16):
                        nc.tensor.matmul(
                            z_ps[:], lhsT=a_fin[:, 16 * k + r:16 * k + r + 1],
                            rhs=vt[:, 512 * r:512 * (r + 1)],
                            start=(k == 0 and r == 0), stop=False,
                        )
                nc.tensor.matmul(z_ps[:], lhsT=a_fin[0:1, 64:65], rhs=v_row[:],
                                 start=False, stop=True)

                z_sb = kp.tile([1, d_sh + 8], F32, tag="zsb", name="z_sb")
                nc.scalar.dma_start(z_sb[0:1, d_sh:d_sh + 8], cc_w_out[:])
                nc.vector.tensor_copy(z_sb[0:1, 0:d_sh], z_ps[:])
                nc.gpsimd.dma_start(z_d[:], z_sb[:])

    # Attach the external-semaphore wait to the first reduce add, now that
    # Tile scheduling is done: HW blocks the DVE queue here until all 7
    # remote writes have landed (each bumps x_recv by 2).
    ins = first_add.ins
    si = ins.sync_info
    ow = list(si.on_wait) if si is not None else []
    ou = list(si.on_update) if si is not None else []
    ow.append(mybir.SyncWait(sync_type="semaphore", id=x_recv.num,
                             wait_mode="sem-ge-imm",
                             wait_value=2 * (n_cores - 1)))
    ins.sync_info = mybir.SyncInfo(on_wait=ow, on_update=ou)

    nc.compile()
    return nc


def make_in_maps(inputs, n_cores=N_CORES, d_sh=D // N_CORES):
    def bf(a):
        return np.ascontiguousarray(np.asarray(a, np.float32).astype(BF16_NP))

    x = bf(inputs["x"])
    wq = bf(inputs["W_query"])
    wk = bf(inputs["W_key"])
    wv = bf(inputs["W_value"])
    kc = bf(inputs["K_cache"])
    vc = bf(inputs["V_cache"])
    in_maps = []
    for i in range(n_cores):
        r0, r1 = d_sh * i, d_sh * (i + 1)
        in_maps.append({
            "x": x,
            "wq": np.ascontiguousarray(wq[r0:r1]),
            "wk": np.ascontiguousarray(wk[r0:r1]),
            "wvt": np.ascontiguousarray(wv[r0:r1].T),
            "kc": np.ascontiguousarray(kc[r0:r1]),
            "vct": np.ascontiguousarray(vc[r0:r1].T),
        })
    return in_maps


def unshard(per_core_z, d_sh=D // N_CORES):
    shards = [np.asarray(zi)[0, :d_sh].reshape(d_sh, 1) for zi in per_core_z]
    return np.concatenate(shards, axis=0).astype(np.float32)


_NC_CACHE = None


def kernel(x, W_query, W_key, W_value, K_cache, V_cache):
    global _NC_CACHE
    if _NC_CACHE is None:
        _NC_CACHE = build()
    nc = _NC_CACHE
    in_maps = make_in_maps(dict(x=x, W_query=W_query, W_key=W_key,
                                W_value=W_value, K_cache=K_cache,
                                V_cache=V_cache))
    res = run_bass_kernel_spmd(nc, in_maps, core_ids=list(range(N_CORES)))
    return unshard([res.results[i]["z"] for i in range(N_CORES)])


# revision 12
# speedup vs baseline: 48.9851x; 1.0271x over previous
"""Trainium2 Bass kernel for a sigmoid-scored attention decode step with KV cache.

Reference computation (all fp32):
    q = W_query @ x.T ; k = W_key @ x.T ; v = W_value @ x.T          # [4096, 1]
    K = [K_cache | k] ; V = [V_cache | v]                            # [4096, 8193]
    a = sigmoid((q.T @ K) / 64)                                      # [1, 8193]
    z = V @ a.T                                                      # [4096, 1]

Sharding: rows of W_q/W_k/K_cache/V_cache are split across 8 NeuronCores (512
rows each). Each core computes its q/k shard and partial scores over its 512
rows of K; the partial score vectors are summed across cores; sigmoid + the
V-weighted sum are then local per shard. Host slices inputs, casts to bf16
(gate is 2e-2; the problem is HBM-bound so halving traffic dominates), and
pre-transposes the V and W_value shards so the z-phase and the current-step v
run on the tensor engine.

v5 (vs v4 178us / v3 200us / v2 210us / v1 233us):
  - the cross-core score reduction is a hand-rolled all-to-all over
    remote_dma_broadcast (one single-dest send per peer, XOR-relative
    addressing) + 7 local DVE adds, replacing the ncfw AllReduce whose
    fixed cost (trigger->output ~50-65us, after a ~45-65us init) dominated
    v3/v4. Partial scores are exchanged in an interleaved [128, 65] layout
    (one xbar transpose-DMA via a DRAM bounce) so the reduce, sigmoid, and
    the PE z-phase lhsT all use the same tile.
  - v = W_v @ x moved to PE: host supplies W_v^T; x is transpose-loaded as
    [128, 32] chunk-columns; 32 accumulating matmuls give v as a [1, 512]
    row directly (drops 18us of DVE TTRs + the DRAM bounce of v).
"""

import sys

for _p in ("/opt/trn_rl_repo", "/root/.axon_site/_ro/trn_rl_repo"):
    if _p not in sys.path:
        sys.path.append(_p)

import ml_dtypes
import numpy as np

import concourse.bacc as bacc
import concourse.tile as tile
from concourse import mybir
from concourse.bass_utils import run_bass_kernel_spmd
from concourse.dve_ops import TENSOR_TENSOR_REDUCE

N_CORES = 8
E = 4096          # embedding dim (contraction for q/k/v)
D = 4096          # output dim
T = 8192          # cached timesteps
F32 = mybir.dt.float32
BF16 = mybir.dt.bfloat16
BF16_NP = ml_dtypes.bfloat16


def build(n_cores=N_CORES, e=E, d_sh=D // N_CORES, t=T):
    nd = d_sh // 128             # partition-chunks per core (4)
    half = t // 2                # score columns per K column-group (4096)
    nr = t // 128                # t-chunks for the PE z phase (64)
    wex = 80                     # exchange tile cols (>= 65, multiple of 16)
    payw = wex * 128             # staging row width (10240)
    ne = e // 128                # e-chunks for the PE v row (32)

    nc = bacc.Bacc("TRN2", target_bir_lowering=False, debug=False,
                   num_devices=n_cores)
    x_d = nc.dram_tensor("x", [1, e], BF16, kind="ExternalInput").ap()
    wq_d = nc.dram_tensor("wq", [d_sh, e], BF16, kind="ExternalInput").ap()
    wk_d = nc.dram_tensor("wk", [d_sh, e], BF16, kind="ExternalInput").ap()
    wvt_d = nc.dram_tensor("wvt", [e, d_sh], BF16, kind="ExternalInput").ap()
    kc_d = nc.dram_tensor("kc", [d_sh, t], BF16, kind="ExternalInput").ap()
    vct_d = nc.dram_tensor("vct", [t, d_sh], BF16, kind="ExternalInput").ap()
    z_d = nc.dram_tensor("z", [1, d_sh + 8], F32, kind="ExternalOutput").ap()

    def chunked(src):
        # [n*128, w] DRAM region -> [128, n, w]: block c holds rows
        # 128c..128c+127. Paired with a [p (c t) -> p c t] view of the tile.
        return src.rearrange("(c p) t -> p c t", p=128)

    def as3d(tile_ap, w):
        return tile_ap.rearrange("p (c t) -> p c t", t=w)

    x_recv = nc.alloc_semaphore("x_recv")
    x_sent = nc.alloc_semaphore("x_sent")

    with tile.TileContext(nc) as tc:
        with (
            tc.tile_pool(name="stream", bufs=6) as sp,       # streamed 2MB tiles
            tc.tile_pool(name="scratch", bufs=1) as scp,     # ttr elementwise outs
            tc.tile_pool(name="keep", bufs=1) as kp,         # persistent tiles
            tc.tile_pool(name="acc", bufs=8) as accp,        # [128,1] accumulators
            tc.tile_pool(name="dram", bufs=1, space="DRAM") as dramp,
        ):
            # --- x: broadcast across partitions + interleaved chunk-columns ---
            bx = kp.tile([128, e], BF16, tag="bx", name="bx")
            nc.sync.dma_start(bx[:], x_d[0:1, :].partition_broadcast(128))
            x_c = kp.tile([128, ne], BF16, tag="xc", name="x_c")
            nc.scalar.dma_start_transpose(
                x_c[:], x_d[0:1, :].rearrange("1 (r p) -> r p", p=128))

            ones_col = kp.tile([128, 1], BF16, tag="onesc", name="ones_col")
            nc.vector.memset(ones_col[:], 1.0)
            # pre-warm the sigmoid ACT table so the load is off the critical path
            warm = kp.tile([1, 1], BF16, tag="warm", name="warm")
            nc.vector.memset(warm[:], 0.0)
            nc.scalar.activation(warm[:], warm[:],
                                 mybir.ActivationFunctionType.Sigmoid,
                                 scale=1.0 / 64.0)

            # partial scores staging row (bf16): [0,t) cache scores, t = qk,
            # rest zero
            s_sb = kp.tile([1, payw], BF16, tag="s", name="s_sb")
            nc.vector.memset(s_sb[0:1, t + 1:payw], 0.0)

            qkv_all = kp.tile([128, 2 * nd], BF16, tag="qkv", name="qkv_all")

            def w_matvec(w_dram, col0):
                # two [128, 2*e] transfers; TTR against bx per col block
                for k in range(2):
                    wt = sp.tile([128, 2 * e], BF16, tag="big",
                                 name=f"wt{col0}_{k}")
                    nc.sync.dma_start(as3d(wt[:], e),
                                      chunked(w_dram[256 * k:256 * (k + 1), :]))
                    for c in range(2):
                        sc = scp.tile([128, e], BF16, tag="sc",
                                      name=f"wsc{col0}_{k}_{c}")
                        nc.vector._custom_dve(
                            TENSOR_TENSOR_REDUCE, out=sc[:],
                            in0=wt[:, e * c:e * (c + 1)], in1=bx[:],
                            s0=0.0, s1=1.0,
                            accum_out=qkv_all[:, col0 + 2 * k + c:col0 + 2 * k + c + 1],
                        )

            w_matvec(wq_d, 0)        # q in cols 0..nd-1
            w_matvec(wk_d, nd)       # k in cols nd..2nd-1

            psq_ctx = tc.tile_pool(name="psq", bufs=1, space="PSUM")
            psq = psq_ctx.__enter__()
            psp_ctx = tc.tile_pool(name="ps", bufs=3, space="PSUM")
            psp = psp_ctx.__enter__()
            psv_ctx = tc.tile_pool(name="psv", bufs=1, space="PSUM")
            psv = psv_ctx.__enter__()

            # --- appended-column partial score q.k (early: it gates the
            # --- exchange together with the score casts) ---
            qk_el = scp.tile([128, nd], BF16, tag="sc", name="qk_el")
            qk_part = accp.tile([128, 1], BF16, tag="acc", name="qk_part")
            nc.vector._custom_dve(
                TENSOR_TENSOR_REDUCE, out=qk_el[:], in0=qkv_all[:, 0:nd],
                in1=qkv_all[:, nd:2 * nd], s0=0.0, s1=1.0,
                accum_out=qk_part[:],
            )
            qk_ps = psq.tile([1, 512], F32, tag="psq", name="qk_ps")
            nc.tensor.matmul(qk_ps[0:1, 0:1], lhsT=ones_col[:],
                             rhs=qk_part[:], start=True, stop=True)
            nc.vector.tensor_copy(s_sb[0:1, t:t + 1], qk_ps[0:1, 0:1])

            # --- partial scores: [1,1024] PSUM tiles, 4 per column group ---
            score_tiles = {}
            for g in range(2):
                for k in range(2):
                    kt = sp.tile([128, 2 * half], BF16, tag="big",
                                 name=f"kt{g}_{k}")
                    nc.sync.dma_start(
                        as3d(kt[:], half),
                        chunked(kc_d[256 * k:256 * (k + 1),
                                     half * g:half * (g + 1)]))
                    for ci in range(2):
                        c = 2 * k + ci
                        for i in range(4):
                            if k == 0 and ci == 0:
                                score_tiles[(g, i)] = psp.tile(
                                    [1, 1024], F32, tag="ps", name=f"ps{g}_{i}")
                            ps = score_tiles[(g, i)]
                            for j in range(2):
                                lo = half * ci + 1024 * i + 512 * j
                                nc.tensor.matmul(
                                    ps[0:1, 512 * j:512 * (j + 1)],
                                    lhsT=qkv_all[:, c:c + 1],
                                    rhs=kt[:, lo:lo + 512],
                                    start=(c == 0), stop=(c == nd - 1),
                                )
                # copy-cast f32 PSUM -> bf16 staging
                for i in range(4):
                    nc.vector.tensor_copy(
                        s_sb[0:1, half * g + 1024 * i:half * g + 1024 * (i + 1)],
                        score_tiles[(g, i)][:])

            # --- exchange: bounce to DRAM, transpose-load interleaved, send to
            # --- all 7 peers, reduce locally ---
            sc_d = dramp.tile([1, payw], BF16, tag="sc_d", name="sc_d")
            nc.gpsimd.dma_start(sc_d[:], s_sb[:])
            a_x = kp.tile([128, wex], BF16, tag="ax", name="a_x")
            nc.scalar.dma_start_transpose(
                a_x[:], sc_d[0:1, :].rearrange("1 (r p) -> r p", p=128))

            recvs = []
            for j in range(1, n_cores):
                rv = kp.tile([128, wex], BF16, tag=f"rv{j}", name=f"rv{j}")
                recvs.append(rv)
                rdests = [None] * 8
                rdests[j] = (0, j)
                nc.gpsimd.remote_dma_broadcast(
                    rv[:], a_x[:], remote_sem=x_recv, local_sem=x_sent,
                    rdests=rdests,
                )
            nc.gpsimd.trigger_dma(count=None)

            # --- warm-up collective AFTER the exchange trigger: its presence
            # --- switches NRT into synchronized multi-core launch (without it
            # --- the 8 cores start milliseconds apart and the exchange eats
            # --- the skew). The ncfw trigger blocks the issuing queue until
            # --- the op completes (~30us after a ~50us one-time init), so it
            # --- must sit behind the exchange preps on gpsimd. Its zero
            # --- output is appended to the z output so it is not DCE'd.
            w_sb = kp.tile([1, 8], F32, tag="warmsb", name="w_sb")
            nc.vector.memset(w_sb[:], 0.0)
            cc_w_in = dramp.tile([1, 8], F32, tag="cc_w_in", name="cc_w_in")
            cc_w_out = dramp.tile([1, 8], F32, tag="cc_w_out", name="cc_w_out")
            nc.gpsimd.dma_start(cc_w_in[:], w_sb[:])
            nc.gpsimd.collective_compute(
                "AllReduce", mybir.AluOpType.add,
                replica_groups=[list(range(n_cores))],
                ins=[cc_w_in.opt()], outs=[cc_w_out.opt()],
            )

            # accumulate the 7 peer contributions (ping-pong). The wait for
            # x_recv >= 14 is attached to the FIRST add post-scheduling (the
            # Tile scheduling simulator cannot model externally-satisfied
            # semaphores and would report a deadlock).
            asums = [kp.tile([128, wex], BF16, tag=f"asum{i}", name=f"asum{i}")
                     for i in range(2)]
            cur = a_x
            first_add = None
            for idx, rv in enumerate(recvs):
                nxt = asums[idx % 2]
                bi = nc.vector.scalar_tensor_tensor(
                    out=nxt[:], in0=cur[:], scalar=1.0, in1=rv[:],
                    op0=mybir.AluOpType.mult, op1=mybir.AluOpType.add,
                )
                if first_add is None:
                    first_add = bi
                cur = nxt
            a_fin = cur
            nc.scalar.activation(a_fin[:, 0:65], a_fin[:, 0:65],
                                 mybir.ActivationFunctionType.Sigmoid,
                                 scale=1.0 / 64.0)

            # --- v = W_v @ x as a [1, 512] row on PE ---
            v_ps = psv.tile([1, d_sh], F32, tag="psv", name="v_ps")
            for k in range(2):
                wvt = sp.tile([128, 16 * d_sh], BF16, tag="big",
                              name=f"wvt{k}")
                nc.sync.dma_start(
                    as3d(wvt[:], d_sh),
                    chunked(wvt_d[2048 * k:2048 * (k + 1), :]))
                for m in range(16):
                    nc.tensor.matmul(
                        v_ps[:], lhsT=x_c[:, 16 * k + m:16 * k + m + 1],
                        rhs=wvt[:, 512 * m:512 * (m + 1)],
                        start=(k == 0 and m == 0), stop=(k == 1 and m == 15),
                    )
            v_row = kp.tile([1, d_sh], BF16, tag="vrow", name="v_row")
            nc.vector.tensor_copy(v_row[:], v_ps[:])

            psv_ctx.__exit__(None, None, None)
            psp_ctx.__exit__(None, None, None)
            psq_ctx.__exit__(None, None, None)

            # --- z = V @ a on PE: 64 accumulating matmuls + appended column ---
            with tc.tile_pool(name="ps2", bufs=1, space="PSUM") as psp2:
                z_ps = psp2.tile([1, d_sh], F32, tag="zps", name="z_ps")
                for k in range(4):
                    vt = sp.tile([128, 16 * d_sh], BF16, tag="big",
                                 name=f"vt{k}")
                    nc.sync.dma_start(
                        as3d(vt[:], d_sh),
                        chunked(vct_d[2048 * k:2048 * (k + 1), :]))
                    for r in range(16):
                        nc.tensor.matmul(
                            z_ps[:], lhsT=a_fin[:, 16 * k + r:16 * k + r + 1],
                            rhs=vt[:, 512 * r:512 * (r + 1)],
                            start=(k == 0 and r == 0), stop=False,
                        )
                nc.tensor.matmul(z_ps[:], lhsT=a_fin[0:1, 64:65], rhs=v_row[:],
                                 start=False, stop=True)

                z_sb = kp.tile([1, d_sh + 8], F32, tag="zsb", name="z_sb")
                nc.scalar.dma_start(z_sb[0:1, d_sh:d_sh + 8], cc_w_out[:])
                nc.vector.tensor_copy(z_sb[0:1, 0:d_sh], z_ps[:])
                nc.sync.dma_start(z_d[:], z_sb[:])

    # Attach the external-semaphore wait to the first reduce add, now that
    # Tile scheduling is done: HW blocks the DVE queue here until all 7
    # remote writes have landed (each bumps x_recv by 2).
    ins = first_add.ins
    si = ins.sync_info
    ow = list(si.on_wait) if si is not None else []
    ou = list(si.on_update) if si is not None else []
    ow.append(mybir.SyncWait(sync_type="semaphore", id=x_recv.num,
                             wait_mode="sem-ge-imm",
                             wait_value=2 * (n_cores - 1)))
    ins.sync_info = mybir.SyncInfo(on_wait=ow, on_update=ou)

    nc.compile()
    return nc


def make_in_maps(inputs, n_cores=N_CORES, d_sh=D // N_CORES):
    def bf(a):
        return np.ascontiguousarray(np.asarray(a, np.float32).astype(BF16_NP))

    x = bf(inputs["x"])
    wq = bf(inputs["W_query"])
    wk = bf(inputs["W_key"])
    wv = bf(inputs["W_value"])
    kc = bf(inputs["K_cache"])
    vc = bf(inputs["V_cache"])
    in_maps = []
    for i in range(n_cores):
        r0, r1 = d_sh * i, d_sh * (i + 1)
        in_maps.append({
            "x": x,
            "wq": np.ascontiguousarray(wq[r0:r1]),
            "wk": np.ascontiguousarray(wk[r0:r1]),
            "wvt": np.ascontiguousarray(wv[r0:r1].T),
            "kc": np.ascontiguousarray(kc[r0:r1]),
            "vct": np.ascontiguousarray(vc[r0:r1].T),
        })
    return in_maps


def unshard(per_core_z, d_sh=D // N_CORES):
    shards = [np.asarray(zi)[0, :d_sh].reshape(d_sh, 1) for zi in per_core_z]
    return np.concatenate(shards, axis=0).astype(np.float32)


_NC_CACHE = None


def kernel(x, W_query, W_key, W_value, K_cache, V_cache):
    global _NC_CACHE
    if _NC_CACHE is None:
        _NC_CACHE = build()
    nc = _NC_CACHE
    in_maps = make_in_maps(dict(x=x, W_query=W_query, W_key=W_key,
                                W_value=W_value, K_cache=K_cache,
                                V_cache=V_cache))
    res = run_bass_kernel_spmd(nc, in_maps, core_ids=list(range(N_CORES)))
    return unshard([res.results[i]["z"] for i in range(N_CORES)])
